# revision 27
# baseline (speedup 1.0000x reference)
"""DGCNN (nn_DGCNN_44478681317800) Bass kernel for 8 Trainium2 NeuronCores.

Strategy:
  - Data-parallel over batch: 16 point clouds -> 2 per core, same NEFF on all
    8 cores (SPMD), different x slices per core.
  - EdgeConv linearization: with w = [wA | wB] acting on [x_j - x_i ; x_i],
        y[o,i,j] = (wA x)[o,j] + ((wB - wA) x)[o,i]  =: U[o,j] + V[o,i]
    Since s>0 and LeakyReLU are monotone, max_j commutes:
        out[o,i] = Prelu(max_{j in knn(i)} sU[o,j] + sV[o,i] + b_o)
    s is folded into U,V weights host-side; b applied via ACT bias.
  - kNN: ranking of neg-dist within row i equals ranking of
        D[i,j] = <x_i, x_j> - ||x_j||^2/2   (row-constant shift/scale dropped)
    computed on PE; exact top-20 per row via 3 rounds of DVE max8 /
    max_index / match_replace (top-24 sorted, first 20 = exact top-20 set).
  - Neighbor gather: GPSIMD indirect_copy on channel-major U (indices shared
    across partitions, staged via DRAM into the wrapped 16-partition layout),
    then DVE grouped reduce_max over k=20.
  - Layers of the two local batches are interleaved so DVE-heavy selection of
    one batch overlaps PE/POOL/ACT work of the other.
"""
import math
from contextlib import ExitStack

import numpy as np

import concourse.bass as bass
from concourse import bacc
import concourse.mybir as mybir
from concourse.tile import TileContext
from concourse.bass_utils import run_bass_kernel_spmd

F32 = mybir.dt.float32
F32R = mybir.dt.float32r
U16 = mybir.dt.uint16
AX = mybir.AxisListType
AF = mybir.ActivationFunctionType

N = 1024
K = 20
NCORES = 8
BPC = 2  # batches per core
LAYERS = [(3, 64), (64, 64), (64, 128), (128, 256)]  # (C_in, O)
NEG_BIG = -3.0e38


# fp32r is e8m11 (11-bit mantissa) -> too coarse for the distance/value path
# (neighbor-set flips); plain fp32 matmuls (4 cyc/row on PE).
def _mm(nc, out, lhsT, rhs, start=True, stop=True):
    nc.tensor.matmul(out, lhsT, rhs, start=start, stop=stop)


def build_nc(reps=1):
    nc = bacc.Bacc("TRN2", target_bir_lowering=False, debug=False)

    x_in = nc.dram_tensor("xloc", [BPC, 3, N], F32, kind="ExternalInput")
    d_lhsU = {}
    d_lhsV = {}
    d_bcol = {}
    for li, (C, O) in enumerate(LAYERS, 1):
        Opad = 128 * math.ceil(O / 128)
        d_lhsU[li] = nc.dram_tensor(f"lhsU{li}", [C, Opad], F32, kind="ExternalInput")
        d_lhsV[li] = nc.dram_tensor(f"lhsV{li}", [C, O], F32, kind="ExternalInput")
        d_bcol[li] = nc.dram_tensor(f"bcol{li}", [O, 1], F32, kind="ExternalInput")
    d_lhsY = nc.dram_tensor("lhsY", [512, 512], F32, kind="ExternalInput")
    d_b5 = nc.dram_tensor("b5col", [512, 1], F32, kind="ExternalInput")

    o_equ = nc.dram_tensor("x_equ", [BPC, 1024], F32, kind="ExternalOutput")
    o_inv = nc.dram_tensor("x_inv", [BPC, 512, N], F32, kind="ExternalOutput")

    with TileContext(nc) as tc, ExitStack() as ctx:
        ep = ctx.enter_context
        constp = ep(tc.tile_pool(name="const", bufs=1))
        wts = ep(tc.tile_pool(name="wts", bufs=1))
        featp = ep(tc.tile_pool(name="feat", bufs=2))
        sqp = ep(tc.tile_pool(name="sq", bufs=2))
        negxxp = ep(tc.tile_pool(name="negxx", bufs=2))
        dsbp = ep(tc.tile_pool(name="dsb", bufs=4))
        selp = ep(tc.tile_pool(name="sel", bufs=4))
        idxrepp = ep(tc.tile_pool(name="idxrep", bufs=2))
        uvp = ep(tc.tile_pool(name="uv", bufs=2))
        gathp = ep(tc.tile_pool(name="gath", bufs=3))
        mxp = ep(tc.tile_pool(name="mx", bufs=2))
        ysbp = ep(tc.tile_pool(name="ysb", bufs=2))
        smallp = ep(tc.tile_pool(name="small", bufs=8))
        psD = ep(tc.tile_pool(name="psD", bufs=5, space="PSUM"))
        psMM = ep(tc.tile_pool(name="psMM", bufs=2, space="PSUM"))
        psXX = ep(tc.tile_pool(name="psXX", bufs=1, space="PSUM"))
        dramp = ep(tc.tile_pool(name="dram", bufs=2, space="DRAM"))

        # fp32 matmuls carry at most ONE sync wait -> every PE operand must be
        # produced by a single processor; "launder" all PE inputs through ACT.
        def act_staged(dram_ap, shape, tag):
            t0 = wts.tile(shape, F32, tag=tag + "_0", name=tag + "_0")
            nc.sync.dma_start(t0[:], dram_ap)
            t = wts.tile(shape, F32, tag=tag, name=tag)
            nc.scalar.copy(t[:], t0[:])
            return t

        ones_st = constp.tile([128, 1], F32, tag="ones_st", name="ones_st")
        nc.vector.memset(ones_st[:], 1.0)
        ones_col = constp.tile([128, 1], F32, tag="ones_col", name="ones_col")
        nc.scalar.copy(ones_col[:], ones_st[:])
        ones_row_st = constp.tile([1, 128], F32, tag="ones_row_st", name="ones_row_st")
        nc.vector.memset(ones_row_st[:], 1.0)
        ones_row = constp.tile([1, 128], F32, tag="ones_row", name="ones_row")
        nc.scalar.copy(ones_row[:], ones_row_st[:])

        # ---- weights to SBUF (once) ----
        wU, wV, wb = {}, {}, {}
        for li, (C, O) in enumerate(LAYERS, 1):
            Opad = 128 * math.ceil(O / 128)
            wU[li] = act_staged(d_lhsU[li][:], [C, Opad], f"wU{li}")
            wV[li] = act_staged(d_lhsV[li][:], [C, O], f"wV{li}")
            wb[li] = [
                act_staged(d_bcol[li][oc * 128:min(O, (oc + 1) * 128), :],
                           [min(O - oc * 128, 128), 1], f"wb{li}_{oc}")
                for oc in range(Opad // 128)
            ]
        # final layer: K-chunks 64,64,128,128,128 (x1,x2,x3,x4a,x4b).
        # Runs in float32r (e8m11): ~1e-4 rel error on x_inv/x_equ only
        # (no kNN downstream), 4x faster on PE.
        def act_staged_r(dram_ap, shape, tag):
            t0 = wts.tile(shape, F32, tag=tag + "_0", name=tag + "_0")
            nc.sync.dma_start(t0[:], dram_ap)
            t = wts.tile(shape, F32R, tag=tag, name=tag)
            nc.scalar.copy(t[:], t0[:])
            return t

        ksplit = [(0, 128), (128, 128), (256, 128), (384, 128)]
        wY = [act_staged_r(d_lhsY[k0:k0 + kc, :], [kc, 512], f"wY{ki}")
              for ki, (k0, kc) in enumerate(ksplit)]
        wb5 = [act_staged(d_b5[ot * 128:(ot + 1) * 128, :], [128, 1], f"wb5_{ot}")
               for ot in range(4)]

        def edge_conv(bi, li, X, xcr_dst):
            C, O = LAYERS[li - 1]
            Opad = 128 * math.ceil(O / 128)
            n_oc = Opad // 128
            # fold -xx/2 into the distance matmul via an aug row; compute-engine
            # partition slices must start 32-aligned -> only C=64 layers qualify
            fold_aug = C == 64

            # xx = sum_c X^2 ; negxx = -xx/2 (1, N)
            X2 = sqp.tile([C, N], F32, tag="sq", name=f"sq{li}_{bi}")
            nc.scalar.square(X2[:], X[:])
            negxx = negxxp.tile([1, N], F32, tag="negxx", name=f"negxx{li}_{bi}")
            for h in range(2):
                xxps = psXX.tile([1, 512], F32, tag="xx", name="xxps")
                _mm(nc, xxps[:], ones_col[:C, :], X2[:, h * 512:(h + 1) * 512])
                nc.scalar.mul(negxx[:, h * 512:(h + 1) * 512], xxps[:], -0.5)

            if fold_aug:
                # Xw = [X; ones] for lhsT, Xm = [X; -xx/2] for the moving side
                Xw = sqp.tile([C + 1, N], F32, tag="Xw", name=f"Xw{li}_{bi}")
                nc.scalar.copy(Xw[:C, :], X[:])
                nc.scalar.activation(Xw[C:C + 1, :], negxx[:], AF.Copy,
                                     bias=1.0, scale=0.0)
                Xm = sqp.tile([C + 1, N], F32, tag="Xm", name=f"Xm{li}_{bi}")
                nc.scalar.copy(Xm[:C, :], X[:])
                nc.scalar.copy(Xm[C:C + 1, :], negxx[:])

            # U = sWA @ X (zero-padded to Opad), V = s(wB-wA) @ X
            Us, Vs = [], []
            for oc in range(n_oc):
                Usb = uvp.tile([128, N], F32, tag="U", name=f"U{li}_{bi}_{oc}")
                for h in range(2):
                    ups = psMM.tile([128, 512], F32, tag="mm", name="ups")
                    _mm(nc, ups[:], wU[li][:, oc * 128:(oc + 1) * 128],
                        X[:, h * 512:(h + 1) * 512])
                    nc.scalar.copy(Usb[:, h * 512:(h + 1) * 512], ups[:])
                Us.append(Usb)
                Oc = min(O - oc * 128, 128)
                Vsb = uvp.tile([Oc, N], F32, tag="V", name=f"V{li}_{bi}_{oc}")
                for h in range(2):
                    vps = psMM.tile([128, 512], F32, tag="mm", name="vps")
                    _mm(nc, vps[:Oc, :], wV[li][:, oc * 128:oc * 128 + Oc],
                        X[:, h * 512:(h + 1) * 512])
                    nc.scalar.copy(Vsb[:, h * 512:(h + 1) * 512], vps[:Oc, :])
                Vs.append(Vsb)

            # distance blocks + exact top-20 selection per point tile
            idx_dram = dramp.tile([8, 128, K], U16, tag="idxd", name=f"idxd{li}_{bi}")
            for pt in range(8):
                Dsb = dsbp.tile([128, N], F32, tag="D", name=f"D{li}_{bi}_{pt}")
                for h in range(2):
                    dps = psD.tile([128, 512], F32, tag="D", name="dps")
                    if fold_aug:
                        _mm(nc, dps[:], Xw[:, pt * 128:(pt + 1) * 128],
                            Xm[:, h * 512:(h + 1) * 512], start=True, stop=True)
                    else:
                        _mm(nc, dps[:], X[:, pt * 128:(pt + 1) * 128],
                            X[:, h * 512:(h + 1) * 512], start=True, stop=False)
                        _mm(nc, dps[:], ones_row[:, :],
                            negxx[:, h * 512:(h + 1) * 512], start=False, stop=True)
                    nc.scalar.copy(Dsb[:, h * 512:(h + 1) * 512], dps[:])
                idxt = selp.tile([128, 24], U16, tag="idxt", name=f"idxt{li}_{bi}_{pt}")
                mx8 = selp.tile([128, 8], F32, tag="mx8", name=f"mx8{li}_{bi}_{pt}")
                for r in range(3):
                    nc.vector.max(out=mx8[:], in_=Dsb[:])
                    nc.vector.max_index(out=idxt[:, r * 8:(r + 1) * 8],
                                        in_max=mx8[:], in_values=Dsb[:])
                    if r < 2:
                        nc.vector.match_replace(out=Dsb[:], in_to_replace=mx8[:],
                                                in_values=Dsb[:], imm_value=NEG_BIG)
                nc.sync.dma_start(idx_dram[pt], idxt[:, :K])

            # Replicated index image. DRAM holds the flat h-order list
            # (h = p*20 + t per ptile). We make the wrapped image's row m
            # simply DRAM[m*160 : (m+1)*160] (contiguous!), which induces the
            # gather list order g = s*16 + m with value idxt[p, t] at
            # h = m*160 + s, i.e. G position g(p, t) = 320*(p%8) + 16*t + p//8.
            idx_rep = idxrepp.tile([128, 1280], U16, tag="idxrep",
                                   name=f"idxrep{li}_{bi}")
            srcv = idx_dram[:].rearrange("pt p t -> (pt p t)").rearrange(
                "(pt m s) -> m pt s", pt=8, m=16)      # (16, 8, 160)
            for c in range(8):
                dstv = idx_rep[c * 16:(c + 1) * 16, :].rearrange(
                    "m (pt s) -> m pt s", pt=8)
                nc.sync.dma_start(dstv, srcv)

            # gather + grouped max + epilogue
            Xn = []
            for oc in range(n_oc):
                Oc = min(O - oc * 128, 128)
                Mx = mxp.tile([128, N], F32, tag="Mx", name=f"Mx{li}_{bi}_{oc}")
                for pt in range(8):
                    G = gathp.tile([128, 128 * K], F32, tag="G",
                                   name=f"G{li}_{bi}_{oc}_{pt}")
                    for q0, qn in [(0, 1024), (1024, 1024), (2048, 512)]:
                        nc.gpsimd.indirect_copy(
                            out=G[:, q0:q0 + qn], data=Us[oc][:],
                            idxs=idx_rep[:, pt * 160 + q0 // 16:
                                         pt * 160 + (q0 + qn) // 16],
                            i_know_ap_gather_is_preferred=True)
                    # G position for point p, neighbor t: 320*(p%8) + 16*t + p//8
                    # Only the first Oc partitions (channels) are meaningful.
                    Gv = G[:Oc, :].rearrange("p (r t a) -> p r a t", t=K, a=16)
                    Mxv = Mx[:Oc, pt * 128:(pt + 1) * 128].rearrange(
                        "p (a r) -> p r a", r=8)
                    nc.vector.reduce_max(out=Mxv, in_=Gv, axis=AX.X)
                nc.vector.tensor_add(Mx[:Oc, :], Mx[:Oc, :], Vs[oc][:])
                xcr, r0 = xcr_dst[oc]
                if li < 4:
                    Xt = featp.tile([Oc, N], F32, tag=f"x{li}_{oc}",
                                    name=f"x{li}_{oc}_{bi}")
                    nc.scalar.activation(Xt[:], Mx[:Oc, :], AF.Prelu,
                                         bias=wb[li][oc][:], scale=1.0, alpha=0.2)
                    # float32r shadow into the packed final-matmul operand
                    nc.scalar.copy(xcr[r0:r0 + Oc, :], Xt[:])
                    Xn.append(Xt)
                else:
                    # x4 feeds only the final matmul: produce f32r directly
                    nc.scalar.activation(xcr[r0:r0 + Oc, :], Mx[:Oc, :], AF.Prelu,
                                         bias=wb[li][oc][:], scale=1.0, alpha=0.2)
            return Xn

        def final_layer(bi, xcrs):
            for ot in range(4):
                ysb = ysbp.tile([128, N], F32, tag="y", name=f"y{bi}_{ot}")
                accs = []
                for h in range(2):
                    yps = psMM.tile([128, 512], F32, tag="mm", name="yps")
                    for ki, fr in enumerate(xcrs):
                        _mm(nc, yps[:], wY[ki][:, ot * 128:(ot + 1) * 128],
                            fr[:, h * 512:(h + 1) * 512],
                            start=(ki == 0), stop=(ki == len(xcrs) - 1))
                    acc = smallp.tile([128, 1], F32, tag="acc",
                                      name=f"acc{bi}_{ot}_{h}")
                    nc.scalar.activation(ysb[:, h * 512:(h + 1) * 512], yps[:],
                                         AF.Prelu, bias=wb5[ot][:], scale=1.0,
                                         alpha=0.2, accum_out=acc[:])
                    accs.append(acc)
                nc.sync.dma_start(o_inv[bi, ot * 128:(ot + 1) * 128, :], ysb[:])
                pmax = smallp.tile([128, 1], F32, tag="pmax", name=f"pmax{bi}_{ot}")
                nc.vector.reduce_max(out=pmax[:], in_=ysb[:], axis=AX.X)
                pavg = smallp.tile([128, 1], F32, tag="pavg", name=f"pavg{bi}_{ot}")
                nc.vector.tensor_add(pavg[:], accs[0][:], accs[1][:])
                nc.vector.tensor_scalar_mul(pavg[:], pavg[:], 1.0 / N)
                nc.sync.dma_start(o_equ[bi, ot * 128:(ot + 1) * 128], pmax[:])
                nc.sync.dma_start(o_equ[bi, 512 + ot * 128:512 + (ot + 1) * 128],
                                  pavg[:])

        for rep in range(reps):
            Xcur = []
            xcrs = []
            for bi in range(BPC):
                X = featp.tile([3, N], F32, tag="x0", name=f"x0_{bi}")
                nc.sync.dma_start(X[:], x_in[bi])
                Xcur.append(X)
                xcrs.append([featp.tile([128, N], F32R, tag=f"xcr{k}",
                                        name=f"xcr{k}_{bi}") for k in range(4)])
            # xcr packing: xcr0 = [x1; x2], xcr1 = x3, xcr2 = x4a, xcr3 = x4b
            dsts = {1: [(0, 0)], 2: [(0, 64)], 3: [(1, 0)], 4: [(2, 0), (3, 0)]}
            for li in range(1, 5):
                for bi in range(BPC):
                    xcr_dst = [(xcrs[bi][ci], r0) for ci, r0 in dsts[li]]
                    Xn = edge_conv(bi, li, Xcur[bi], xcr_dst)
                    if li < 4:
                        Xcur[bi] = Xn[0]
            for bi in range(BPC):
                final_layer(bi, xcrs[bi])

    nc.compile()
    return nc


def _prep_weights(inputs):
    wm = {}
    for li, (C, O) in enumerate(LAYERS, 1):
        w = inputs[f"w{li}"].astype(np.float32)
        s = inputs[f"s{li}"].astype(np.float32)
        b = inputs[f"b{li}"].astype(np.float32)
        wA, wB = w[:, :C], w[:, C:]
        Opad = 128 * math.ceil(O / 128)
        lhsU = np.zeros((C, Opad), np.float32)
        lhsU[:, :O] = (s[:, None] * wA).T
        wm[f"lhsU{li}"] = lhsU
        wm[f"lhsV{li}"] = np.ascontiguousarray((s[:, None] * (wB - wA)).T)
        wm[f"bcol{li}"] = b[:, None].copy()
    wm["lhsY"] = np.ascontiguousarray(
        (inputs["s5"].astype(np.float32)[:, None] * inputs["w5"].astype(np.float32)).T)
    wm["b5col"] = inputs["b5"].astype(np.float32)[:, None].copy()
    return wm


_CACHED = {}
LAST_EXEC_NS = None


def kernel(**inputs):
    x = np.ascontiguousarray(inputs["x"], dtype=np.float32)
    assert x.shape == (16, 3, N)
    wm = _prep_weights(inputs)

    if "nc" not in _CACHED:
        _CACHED["nc"] = build_nc()
    nc = _CACHED["nc"]

    in_maps = []
    for c in range(NCORES):
        m = {"xloc": np.ascontiguousarray(x[c * BPC:(c + 1) * BPC])}
        m.update(wm)
        in_maps.append(m)

    res = run_bass_kernel_spmd(nc, in_maps, core_ids=list(range(NCORES)))
    equ = np.concatenate([r["x_equ"] for r in res.results], axis=0)
    inv = np.concatenate([r["x_inv"] for r in res.results], axis=0)
    return equ.astype(np.float32), inv.astype(np.float32)
